# revision 7
# baseline (speedup 1.0000x reference)
"""VQ codebook forward (nn_Codebook) on 8 TRN2 NeuronCores.

Sharding: data-parallel over the 65536 tokens (8192/core, host-side slicing);
codebook/projection weights replicated. Device computes, per 512-token tile:
  z [tok,512] --PE transpose--> zT (f32r) --matmul1 (TF32)--> z_e^T psum
  --ACT copy (f32r)--> z_e sbuf --matmul2 (TF32)--> scores psum [tok,1024]
  DVE: st = scores + (-0.5||c||^2)  (bias pre-replicated from host)
       m = rowmax(st);  sum = SUM_j [st_j >= m-theta] * (j + 16384)
  => sum = C*16384 + S: S = argmax index, C = count of near-max codes
  gather z_out rows from host-precomputed cb_up = codebook @ w_up.T (fp32)
Host decodes code=S, repairs the few C>1 (ambiguous) tokens with exact fp32
scoring, computes the per-batch losses from the m / ||z_e||^2 partial sums.
"""
import numpy as np
from contextlib import ExitStack

import concourse.bass as bass
import concourse.tile as tile
from concourse import bacc, mybir
from concourse.bass_utils import run_bass_kernel_spmd
from concourse.masks import make_identity
from concourse import dve_ops
from concourse.dve_spec import Spec, Src0, Src1, C0, C1, Idx, Zero, lower, maxx
from concourse.dve_uop import DveOpSpec


def _register_vq_ops():
    """Fused DVE op: out = in0 + in1 (psum scores + bias row),
    accum_out = max(c0, max(out)) — one pass instead of TT + reduce_max."""
    if any(op.name == "VQ_BIAS_MAX" for op in dve_ops.OPS):
        return next(op for op in dve_ops.OPS if op.name == "VQ_BIAS_MAX")

    def _ref(in0, in1, c0, c1, c2):
        b = (in0.astype(np.float32) + in1).astype(np.float32)
        acc = np.maximum(
            np.float32(c0),
            b.reshape(b.shape[0], -1).max(-1, keepdims=True).astype(np.float32),
        )
        return b, acc

    spec = Spec(body=Src0 + Src1, accum=maxx, accum_init=C0, reference=_ref)
    opcode = dve_ops._CUSTOM_DVE_ROW_BASE + len(dve_ops.OPS)
    shas = {}
    for ver in ("v3", "v4"):
        try:
            u = lower(spec, ver=ver)
            shas[ver] = DveOpSpec(
                name="VQ_BIAS_MAX", opcode=opcode, uops=u, rd1_en=True
            ).sha(ver)
        except Exception:
            pass
    op = dve_ops.DveOp("VQ_BIAS_MAX", spec, subdim=False, uops_sha=shas)
    dve_ops.OPS.append(op)
    dve_ops.CUSTOM_DVE_SPECS[op.name] = spec
    dve_ops._SUB_OPCODE_FOR_NAME[op.name] = opcode
    return op


VQ_BIAS_MAX = _register_vq_ops()


def _register_vq_idx():
    """Single-src fused pass-2: accum_out = sum((Src0 >= c0) * (Idx + c1)).
    Single tensor read -> 2x_2P eligible (perf_en)."""
    name = "VQ_IDX_SUM"
    for op in dve_ops.OPS:
        if op.name == name:
            return op

    def _ref(in0, in1, c0, c1, c2):
        b = ((in0.astype(np.float32) >= np.float32(c0)).astype(np.float32)
             * (np.arange(in0.shape[-1], dtype=np.float32)[None, :]
                .repeat(in0.shape[0], 0).reshape(in0.shape) + np.float32(c1)))
        acc = b.reshape(b.shape[0], -1).sum(-1, keepdims=True).astype(np.float32)
        return b.astype(np.float32), acc

    spec = Spec(body=(Src0 >= C0) * (Idx + C1), accum=__import__("operator").add,
                accum_init=Zero, reference=_ref)
    opcode = dve_ops._CUSTOM_DVE_ROW_BASE + len(dve_ops.OPS)
    shas = {}
    for ver in ("v3", "v4"):
        try:
            u = lower(spec, ver=ver)
            shas[ver] = DveOpSpec(name=name, opcode=opcode, uops=u,
                                  rd1_en=False).sha(ver)
        except Exception:
            return None
    op = dve_ops.DveOp(name, spec, subdim=False, uops_sha=shas,
                       perf_en={"v3": True, "v4": True})
    dve_ops.OPS.append(op)
    dve_ops.CUSTOM_DVE_SPECS[op.name] = spec
    dve_ops._SUB_OPCODE_FOR_NAME[op.name] = opcode
    return op


VQ_IDX_SUM = _register_vq_idx()

P = 128
D = 512          # input dim
E = 256          # codebook dim
NCODE = 1024
NCORES = 8
TOK_PER_CORE = 8192
TILE_TOK = 512
NTILES = TOK_PER_CORE // TILE_TOK     # 16
NGROUP = TOK_PER_CORE // P            # 64 groups of 128 tokens
THETA = 0.05
BIG = 16384.0

f32 = mybir.dt.float32
f32r = mybir.dt.float32r
i32 = mybir.dt.int32

_CACHE = {}


def build():
    nc = bacc.Bacc("TRN2", target_bir_lowering=False, debug=False)

    z_in = nc.dram_tensor("z_in", [TOK_PER_CORE, D], f32, kind="ExternalInput")
    wdT_in = nc.dram_tensor("wdT_in", [P, 4 * E], f32, kind="ExternalInput")
    cbT_in = nc.dram_tensor("cbT_in", [P, 2 * NCODE], f32, kind="ExternalInput")
    bias_in = nc.dram_tensor("bias_in", [P, NCODE], f32, kind="ExternalInput")
    iotaB_in = nc.dram_tensor("iotaB_in", [P, NCODE], f32, kind="ExternalInput")
    cbup_in = nc.dram_tensor("cbup_in", [NCODE, D], f32, kind="ExternalInput")

    zo_out = nc.dram_tensor("zo_out", [TOK_PER_CORE, D], f32, kind="ExternalOutput")
    sums_out = nc.dram_tensor("sums_out", [P, NGROUP], f32, kind="ExternalOutput")
    m_out = nc.dram_tensor("m_out", [P, NGROUP], f32, kind="ExternalOutput")
    zn_out = nc.dram_tensor("zn_out", [P, 2 * NTILES], f32, kind="ExternalOutput")

    with tile.TileContext(nc) as tc, ExitStack() as ctx:
        const = ctx.enter_context(tc.tile_pool(name="const", bufs=1))
        sbuf = ctx.enter_context(tc.tile_pool(name="sbuf", bufs=2))
        strip = ctx.enter_context(tc.tile_pool(name="strip", bufs=1))
        ps_zt = ctx.enter_context(tc.tile_pool(name="ps_zt", bufs=1, space="PSUM"))
        ps_ze = ctx.enter_context(tc.tile_pool(name="ps_ze", bufs=1, space="PSUM"))
        ps_s = ctx.enter_context(tc.tile_pool(name="ps_s", bufs=2, space="PSUM"))

        # ---------- constants ----------
        ident = const.tile([P, P], f32, tag="ident")
        make_identity(nc, ident[:])

        wdT_f = const.tile([P, 4, E], f32, tag="wdTf")
        nc.sync.dma_start(wdT_f[:], wdT_in[:].rearrange("p (k e) -> p k e", k=4))
        wdT_r = const.tile([P, 4, E], f32r, tag="wdTr")
        nc.scalar.copy(wdT_r[:], wdT_f[:])

        cbT_f = const.tile([P, 2, NCODE], f32, tag="cbTf")
        nc.sync.dma_start(cbT_f[:], cbT_in[:].rearrange("p (k j) -> p k j", k=2))
        cbT_r = const.tile([P, 2, NCODE], f32r, tag="cbTr")
        nc.scalar.copy(cbT_r[:], cbT_f[:])

        bias_rep = const.tile([P, NCODE], f32, tag="bias")
        nc.sync.dma_start(bias_rep[:], bias_in[:])
        iotaB = const.tile([P, NCODE], f32, tag="iotaB")
        nc.sync.dma_start(iotaB[:], iotaB_in[:])

        # strips accumulated across the whole core
        m_strip = strip.tile([P, NGROUP], f32, tag="mstrip")
        sums_strip = strip.tile([P, NGROUP], f32, tag="sstrip")
        zn_strip = strip.tile([P, 2 * NTILES], f32, tag="znstrip")

        for i in range(NTILES):
            # ---------- load z tile ----------
            z_t = sbuf.tile([P, 4, D], f32, tag="z")
            nc.sync.dma_start(
                z_t[:],
                z_in[i * TILE_TOK:(i + 1) * TILE_TOK, :]
                .rearrange("(q p) d -> p q d", p=P),
            )

            # ---------- transpose z (PE, fp32) + round to f32r (ACT) ----------
            zT_sb = sbuf.tile([P, 4, TILE_TOK], f32r, tag="zT")
            for r in range(2):  # two rounds of 2 d-chunks (psum budget)
                zT_ps = ps_zt.tile([P, 2, TILE_TOK], f32, tag="zTps", bufs=1)
                for k in range(2):
                    kk = 2 * r + k
                    for q in range(4):
                        nc.tensor.transpose(
                            zT_ps[:, k, q * P:(q + 1) * P],
                            z_t[:, q, kk * P:(kk + 1) * P],
                            ident[:],
                        )
                nc.scalar.copy(zT_sb[:, 2 * r:2 * r + 2, :], zT_ps[:])

            # ---------- matmul1: z_e^T [2x128 e, 512 tok] ----------
            ze_ps = ps_ze.tile([P, 2, TILE_TOK], f32, tag="zeps")
            for h in range(2):
                for k in range(4):
                    nc.tensor.matmul(
                        ze_ps[:, h, :],
                        wdT_r[:, k, h * P:(h + 1) * P],
                        zT_sb[:, k, :],
                        start=(k == 0), stop=(k == 3),
                    )
            ze_sb = sbuf.tile([P, 2, TILE_TOK], f32r, tag="ze")
            nc.scalar.copy(ze_sb[:], ze_ps[:])
            # znorm partial sums (ACT square + accumulate over tokens)
            sq_scr = sbuf.tile([P, 2, TILE_TOK], f32, tag="sqscr")
            for h in range(2):
                nc.scalar.activation(
                    sq_scr[:, h, :], ze_ps[:, h, :],
                    mybir.ActivationFunctionType.Square,
                    accum_out=zn_strip[:, 2 * i + h:2 * i + h + 1],
                )

            # ---------- per 128-token group: scores + argmax ----------
            ofs_t = sbuf.tile([P, 4], i32, tag="ofs")
            for q in range(4):
                g = 4 * i + q
                s_ps = ps_s.tile([P, NCODE], f32, tag="sps")
                for nh in range(2):
                    for k in range(2):
                        nc.tensor.matmul(
                            s_ps[:, nh * 512:(nh + 1) * 512],
                            ze_sb[:, k, q * P:(q + 1) * P],
                            cbT_r[:, k, nh * 512:(nh + 1) * 512],
                            start=(k == 0), stop=(k == 1),
                        )
                # pass 1 (fused custom DVE): st = scores + bias; m = rowmax
                st_sb = sbuf.tile([P, NCODE], f32, tag="st")
                nc.vector._custom_dve(
                    VQ_BIAS_MAX, out=st_sb[:], in0=s_ps[:], in1=bias_rep[:],
                    s0=-3.0e38, accum_out=m_strip[:, g:g + 1],
                )
                # m - theta on ACT (keeps DVE light)
                mth = sbuf.tile([P, 1], f32, tag="mth")
                nc.scalar.activation(
                    mth[:], m_strip[:, g:g + 1],
                    mybir.ActivationFunctionType.Copy, bias=-THETA,
                )
                # pass 2: sum of (idx+BIG) over near-max codes
                stt_scr = sbuf.tile([P, NCODE], f32, tag="sttscr")
                if VQ_IDX_SUM is not None:
                    nc.vector._custom_dve(
                        VQ_IDX_SUM, out=stt_scr[:], in0=st_sb[:],
                        s0=mth[:], s1=BIG,
                        accum_out=sums_strip[:, g:g + 1],
                    )
                else:
                    nc.vector.scalar_tensor_tensor(
                        out=stt_scr[:], in0=st_sb[:], scalar=mth[:], in1=iotaB[:],
                        op0=mybir.AluOpType.is_ge, op1=mybir.AluOpType.mult,
                        accum_out=sums_strip[:, g:g + 1],
                    )
                # offsets = int(sum) & 16383
                nc.vector.tensor_copy(ofs_t[:, q:q + 1], sums_strip[:, g:g + 1])
                nc.vector.tensor_scalar(
                    out=ofs_t[:, q:q + 1], in0=ofs_t[:, q:q + 1],
                    scalar1=16383, scalar2=None,
                    op0=mybir.AluOpType.bitwise_and,
                )

            # ---------- gather z_out rows from cb_up ----------
            zo_sb = sbuf.tile([P, 4, D], f32, tag="zo")
            for q in range(4):
                nc.gpsimd.indirect_dma_start(
                    out=zo_sb[:, q, :], out_offset=None, in_=cbup_in[:],
                    in_offset=bass.IndirectOffsetOnAxis(ap=ofs_t[:, q:q + 1], axis=0),
                )
            nc.sync.dma_start(
                zo_out[i * TILE_TOK:(i + 1) * TILE_TOK, :]
                .rearrange("(q p) d -> p q d", p=P),
                zo_sb[:],
            )

        # ---------- strips out ----------
        nc.sync.dma_start(sums_out[:], sums_strip[:])
        nc.sync.dma_start(m_out[:], m_strip[:])
        nc.sync.dma_start(zn_out[:], zn_strip[:])

    nc.finalize()
    return nc


def kernel(z, codebook, w_down, w_up):
    z = np.ascontiguousarray(np.asarray(z, dtype=np.float32))
    codebook = np.asarray(codebook, dtype=np.float32)
    w_down = np.asarray(w_down, dtype=np.float32)
    w_up = np.asarray(w_up, dtype=np.float32)
    b, t, d = z.shape
    zf = z.reshape(-1, D)

    # host weight prep (replicated across cores)
    wdT = np.ascontiguousarray(w_down.T)                       # [512, 256]
    wdT_np = np.ascontiguousarray(
        wdT.reshape(4, P, E).transpose(1, 0, 2).reshape(P, 4 * E))
    cbT = np.ascontiguousarray(codebook.T)                     # [256, 1024]
    cbT_np = np.ascontiguousarray(
        cbT.reshape(2, P, NCODE).transpose(1, 0, 2).reshape(P, 2 * NCODE))
    cnorm = (codebook.astype(np.float32) ** 2).sum(1)          # [1024]
    bias_np = np.tile((-0.5 * cnorm)[None, :], (P, 1)).astype(np.float32)
    iotaB_np = np.tile((np.arange(NCODE) + BIG)[None, :], (P, 1)).astype(np.float32)
    cb_up = np.ascontiguousarray(codebook @ w_up.T)            # [1024, 512] fp32

    if "nc" not in _CACHE:
        _CACHE["nc"] = build()
    nc = _CACHE["nc"]

    in_maps = []
    for c in range(NCORES):
        in_maps.append({
            "z_in": zf[c * TOK_PER_CORE:(c + 1) * TOK_PER_CORE],
            "wdT_in": wdT_np, "cbT_in": cbT_np, "bias_in": bias_np,
            "iotaB_in": iotaB_np, "cbup_in": cb_up,
        })

    try:
        res = run_bass_kernel_spmd(nc, in_maps, core_ids=list(range(NCORES)),
                                   trace=bool(_CACHE.get("trace", False)))
    except ModuleNotFoundError:
        res = run_bass_kernel_spmd(nc, in_maps, core_ids=list(range(NCORES)))
    results = res.results
    _CACHE["last_exec_ns"] = res.exec_time_ns
    _CACHE["last_trace"] = res.instructions_and_trace

    # ---------- host assembly ----------
    z_out = np.empty((b * t, D), dtype=np.float32)
    code = np.empty(b * t, dtype=np.int64)
    m_all = np.empty(b * t, dtype=np.float64)
    zn_sum = np.zeros(2 * NCORES, dtype=np.float64)  # per batch

    ambig_tokens = []
    for c in range(NCORES):
        r = results[c]
        lo = c * TOK_PER_CORE
        z_out[lo:lo + TOK_PER_CORE] = r["zo_out"]
        sums = r["sums_out"].astype(np.float64)      # [128, 64]
        C = np.floor(sums / BIG)
        S = sums - C * BIG
        # token t = lo + g*128 + p
        code_c = S.T.reshape(-1).astype(np.int64)    # [8192] token-major
        code[lo:lo + TOK_PER_CORE] = code_c
        m_all[lo:lo + TOK_PER_CORE] = r["m_out"].astype(np.float64).T.reshape(-1)
        amb = np.nonzero((C != 1).T.reshape(-1))[0]
        ambig_tokens.extend((lo + amb).tolist())
        # znorm partials: zn_out [128, 2*NTILES]; tile i cols 2i,2i+1; batch = i//8
        zn = r["zn_out"].astype(np.float64)
        for bb in range(2):
            zn_sum[2 * c + bb] = zn[:, bb * NTILES:(bb + 1) * NTILES].sum()

    # exact repair of ambiguous tokens (and any C==0 paranoia)
    if ambig_tokens:
        idx = np.asarray(ambig_tokens)
        ze_amb = zf[idx] @ w_down.T                   # fp32, matches reference
        sc = ze_amb @ codebook.T - 0.5 * cnorm[None, :]
        new_code = sc.argmax(1)
        code[idx] = new_code
        z_out[idx] = cb_up[new_code]
        m_all[idx] = sc[np.arange(len(idx)), new_code]

    # losses: mean((z_q - z_e)^2) per batch = (sum||z_e||^2 - 2*sum m)/(t*E)
    # tokens are contiguous per batch: batch k covers [k*4096, (k+1)*4096)
    m_per_batch = m_all.reshape(b, t).sum(1)
    loss = (zn_sum - 2.0 * m_per_batch) / (t * E)
    loss = loss.astype(np.float32)

    return (
        z_out.reshape(b, t, D),
        loss.copy(),
        loss.copy(),
        code.reshape(b, t).astype(np.int32),
    )
